# revision 5
# baseline (speedup 1.0000x reference)
"""GQA causal attention layer (QKV proj + NeoX RoPE + softmax attention + o_proj)
for Trainium2, tensor-parallel over heads across 8 NeuronCores.

Problem shapes (hardcoded): B=1, S=2048, HID=2048, NH=32, NKV=8, HD=64.
Per core c: 4 query heads (4c..4c+3) + 1 kv head (c).

v2 (bf16, packed scores, pipelined):
  - All SBUF operands bf16 (PSUM accumulation f32): halves DMA, enables FWL
    weight loads and 2x DVE modes. Accuracy ~0.3% << 2e-2 gate.
  - GQA: all 4 q heads share one kv head, so score matmuls for the even head
    (q rows 0:64, k dup rows 0:64) and odd head (rows 64:128) are emitted as
    adjacent 64x128-mode matmuls at row-tile positions (0,0)/(64,0) -> they
    execute CONCURRENTLY on the PE array (2 cols/cycle aggregate).
  - v transposed via XBAR DMA transpose (no PE transposes, no extra PSUM).
  - Single interleaved emission: qkv chunks, attention (j, head-pair) blocks,
    and o_proj m-chunks share one dependency-scheduled stream so the ~60us
    of ACT exp work hides under PE work. PSUM: ste(2)+sto(2)+pv(2)+qkv/o(2).
  - Per (j, pair): even head PV accumulates in the i-loop (pt ring);
    odd head PV replays from saved pt tiles through the same psum ring.

Host transposes x, pre-slices per-core weights, converts to bf16, and sums
the 8 partial yT outputs (o_proj contraction is split across cores).
"""

import numpy as np
import ml_dtypes

import concourse.bass as bass
import concourse.mybir as mybir
import concourse.tile as tile
from concourse import bacc

B, S, HID = 1, 2048, 2048
NH, NKV, HD = 32, 8, 64
NCORES = 8
ROPE_BASE = 10000.0
SCALE = 1.0 / np.sqrt(HD)   # 0.125
NEG = -1e9

F32 = mybir.dt.float32
BF16 = mybir.dt.bfloat16
NPBF = ml_dtypes.bfloat16

KT = S // 128               # 16 k-position tiles of 128
MC = 512                    # qkv m-chunk
NMC = S // MC               # 4
QCHUNK = 1024               # attention q-chunk
NQC = S // QCHUNK           # 2
EXPF = mybir.ActivationFunctionType.Exp


def _chunks(total, step=512):
    out = []
    o = 0
    while o < total:
        out.append((o, min(step, total - o)))
        o += step
    return out


def build_kernel(passes=1):
    nc = bacc.Bacc("TRN2", target_bir_lowering=False, debug=False,
                   num_devices=NCORES)

    xT = nc.dram_tensor("xT", [HID, S], BF16, kind="ExternalInput").ap()
    w_stat = nc.dram_tensor("w_stat", [HID, 384], BF16, kind="ExternalInput").ap()
    w_o = nc.dram_tensor("w_o", [256, HID], BF16, kind="ExternalInput").ap()
    Cr = nc.dram_tensor("C", [128, S], BF16, kind="ExternalInput").ap()
    Sr = nc.dram_tensor("Sn", [128, S], BF16, kind="ExternalInput").ap()
    maskneg = nc.dram_tensor("maskneg", [128, 128], F32, kind="ExternalInput").ap()
    yT = nc.dram_tensor("yT", [HID, S], BF16, kind="ExternalOutput").ap()

    with tile.TileContext(nc) as tc:
      for _pass in range(passes):
        with (
            tc.tile_pool(name="pers", bufs=1) as pers,
            tc.tile_pool(name="wtp", bufs=KT) as wtp,
            tc.tile_pool(name="vaugp", bufs=1) as vaugp,
            tc.tile_pool(name="xp", bufs=24) as xp,
            tc.tile_pool(name="qkvsb", bufs=2) as qkvsb,
            tc.tile_pool(name="swp", bufs=2) as swp,
            tc.tile_pool(name="rtmp", bufs=2) as rtmp,
            tc.tile_pool(name="ptAp", bufs=4) as ptAp,
            tc.tile_pool(name="ptBp", bufs=KT) as ptBp,
            tc.tile_pool(name="recp", bufs=2) as recp,
            tc.tile_pool(name="otp", bufs=2) as otp,
            tc.tile_pool(name="ysbp", bufs=4) as ysbp,
            tc.tile_pool(name="stps", bufs=1, space="PSUM") as stps,
            tc.tile_pool(name="pvps", bufs=1, space="PSUM") as pvps,
        ):
            # ---- persistent tiles ----
            qr = [pers.tile([128, S], BF16, tag=f"qr{p}", name=f"qr{p}")
                  for p in range(2)]
            # kr rows 64:128 = roped kT; rows 0:64 = DMA duplicate (even heads)
            kr = pers.tile([128, S], BF16, tag="kr")
            outstat = [pers.tile([128, S], BF16, tag=f"os{p}", name=f"os{p}")
                       for p in range(2)]
            wo_sb = [pers.tile([128, HID], BF16, tag=f"wo{p}", name=f"wo{p}")
                     for p in range(2)]
            mneg = pers.tile([128, 128], F32, tag="mneg")
            Ct = pers.tile([128, S], BF16, tag="Ct")
            St = pers.tile([128, S], BF16, tag="St")
            vaug = [vaugp.tile([128, 128], BF16, tag=f"va{i}", name=f"va{i}")
                    for i in range(KT)]

            nc.scalar.dma_start(mneg, maskneg)
            nc.scalar.dma_start(Ct, Cr)
            nc.scalar.dma_start(St, Sr)
            for p in range(2):
                nc.scalar.dma_start(wo_sb[p], w_o[128 * p:128 * (p + 1), :])
            for i in range(KT):
                nc.gpsimd.memset(vaug[i][:, 64:128], 1.0)
            wt = []
            for k in range(KT):
                w = wtp.tile([128, 384], BF16, tag="w", name=f"w{k}")
                nc.gpsimd.dma_start(w, w_stat[128 * k:128 * (k + 1), :])
                wt.append(w)

            def emit_qkv_chunk(c, qkvps):
                """w_stat.T @ x for m-chunk c, + RoPE into qr/kr, + v->vaug."""
                m0 = MC * c
                xts = []
                for k in range(KT):
                    xt = xp.tile([128, MC], BF16, tag="x", name=f"x{c}_{k}")
                    eng = nc.sync if k % 2 == 0 else nc.scalar
                    eng.dma_start(xt, xT[128 * k:128 * (k + 1), m0:m0 + MC])
                    xts.append(xt)
                for n in range(3):
                    ps = qkvps.tile([128, MC], F32, tag="qkv",
                                    name=f"qkvps{c}_{n}")
                    for k in range(KT):
                        nc.tensor.matmul(ps, wt[k][:, 128 * n:128 * (n + 1)],
                                         xts[k], start=(k == 0),
                                         stop=(k == KT - 1))
                    qn = qkvsb.tile([128, MC], BF16, tag=f"q{n}",
                                    name=f"qn{c}_{n}")
                    nc.vector.tensor_copy(qn, ps)
                    # RoPE (NeoX rotate-halves via 32-row swap + cos/sin mulsum)
                    r0, r1 = (0, 128) if n < 2 else (64, 128)
                    sw = swp.tile([128, MC], BF16, tag="sw", name=f"sw{c}_{n}")
                    for g in range(r0 // 32, r1 // 32, 2):
                        nc.gpsimd.dma_start(sw[32 * g:32 * g + 32, :],
                                            qn[32 * g + 32:32 * g + 64, :])
                        nc.gpsimd.dma_start(sw[32 * g + 32:32 * g + 64, :],
                                            qn[32 * g:32 * g + 32, :])
                    t1 = rtmp.tile([128, MC], BF16, tag="t1", name=f"t1_{c}{n}")
                    t2 = rtmp.tile([128, MC], BF16, tag="t2", name=f"t2_{c}{n}")
                    nc.vector.tensor_mul(t1[r0:r1, :], qn[r0:r1, :],
                                         Ct[r0:r1, m0:m0 + MC])
                    nc.vector.tensor_mul(t2[r0:r1, :], sw[r0:r1, :],
                                         St[r0:r1, m0:m0 + MC])
                    dst = qr[n] if n < 2 else kr
                    nc.vector.tensor_add(dst[r0:r1, m0:m0 + MC],
                                         t1[r0:r1, :], t2[r0:r1, :])
                    if n == 2:
                        nc.gpsimd.dma_start(kr[0:64, m0:m0 + MC],
                                            kr[64:128, m0:m0 + MC])
                        for ii in range(4 * c, 4 * (c + 1)):
                            off = 128 * ii - m0
                            nc.sync.dma_start_transpose(
                                vaug[ii][:, 0:64], qn[0:64, off:off + 128])

            def emit_attn(j, p):
                """Attention q-chunk j for head pair p (heads 2p, 2p+1)."""
                jc0 = QCHUNK * j
                ilast = 8 * (j + 1) - 1
                pv = pvps.tile([128, QCHUNK], F32, tag="pv", name=f"pve{j}_{p}")
                ptBs = []
                for i in range(8 * (j + 1)):
                    qstart = max(jc0, 128 * i)
                    qlen = QCHUNK * (j + 1) - qstart
                    qoff = qstart - jc0
                    ste = stps.tile([128, QCHUNK], F32, tag="ste",
                                    name=f"ste{j}_{p}_{i}")
                    sto = stps.tile([128, QCHUNK], F32, tag="sto",
                                    name=f"sto{j}_{p}_{i}")
                    for (c0, cl) in _chunks(qlen):
                        nc.tensor.matmul(
                            ste[:, c0:c0 + cl],
                            kr[0:64, 128 * i:128 * (i + 1)],
                            qr[p][0:64, qstart + c0:qstart + c0 + cl],
                            start=True, stop=True)
                        nc.tensor.matmul(
                            sto[:, c0:c0 + cl],
                            kr[64:128, 128 * i:128 * (i + 1)],
                            qr[p][64:128, qstart + c0:qstart + c0 + cl],
                            start=True, stop=True)
                    if 128 * i >= jc0:
                        nc.vector.tensor_add(ste[:, 0:128], ste[:, 0:128], mneg)
                        nc.vector.tensor_add(sto[:, 0:128], sto[:, 0:128], mneg)
                    ptA = ptAp.tile([128, QCHUNK], BF16, tag="ptA",
                                    name=f"ptA{j}_{p}_{i}")
                    ptB = ptBp.tile([128, QCHUNK], BF16, tag="ptB",
                                    name=f"ptB{j}_{p}_{i}")
                    nc.scalar.activation(ptA[:, 0:qlen], ste[:, 0:qlen],
                                         EXPF, scale=SCALE)
                    nc.scalar.activation(ptB[:, 0:qlen], sto[:, 0:qlen],
                                         EXPF, scale=SCALE)
                    ptBs.append((ptB, qoff, qlen))
                    for (c0, cl) in _chunks(qlen):
                        nc.tensor.matmul(
                            pv[:, qoff + c0:qoff + c0 + cl],
                            vaug[i], ptA[:, c0:c0 + cl],
                            start=(i == 0), stop=(i == ilast))
                # normalize even head -> outstat[p] rows 0:64
                rc = recp.tile([128, QCHUNK], F32, tag="rc", name=f"rch{j}_{p}")
                nc.vector.tensor_copy(rc[64:128, :], pv[64:128, :])
                sums = recp.tile([64, QCHUNK], F32, tag="sums",
                                 name=f"sme{j}_{p}")
                nc.gpsimd.dma_start(sums, rc[64:128, :])
                rec0 = recp.tile([64, QCHUNK], F32, tag="rec",
                                 name=f"rce{j}_{p}")
                nc.vector.reciprocal_approx_fast(rec0, sums)
                nc.vector.tensor_mul(outstat[p][0:64, jc0:jc0 + QCHUNK],
                                     pv[0:64, :], rec0)
                # odd head PV replay from saved pt tiles (same psum ring)
                pvo = pvps.tile([128, QCHUNK], F32, tag="pv", name=f"pvo{j}_{p}")
                for i, (ptB, qoff, qlen) in enumerate(ptBs):
                    for (c0, cl) in _chunks(qlen):
                        nc.tensor.matmul(
                            pvo[:, qoff + c0:qoff + c0 + cl],
                            vaug[i], ptB[:, c0:c0 + cl],
                            start=(i == 0), stop=(i == ilast))
                rco = recp.tile([128, QCHUNK], F32, tag="rc",
                                name=f"rcho{j}_{p}")
                nc.vector.tensor_copy(rco[64:128, :], pvo[64:128, :])
                sums_o = recp.tile([64, QCHUNK], F32, tag="sums",
                                   name=f"smo{j}_{p}")
                nc.gpsimd.dma_start(sums_o, rco[64:128, :])
                rec_o = recp.tile([64, QCHUNK], F32, tag="rec",
                                  name=f"rco{j}_{p}")
                nc.vector.reciprocal_approx_fast(rec_o, sums_o)
                ot = otp.tile([64, QCHUNK], BF16, tag="ot", name=f"ot{j}_{p}")
                nc.vector.tensor_mul(ot, pvo[0:64, :], rec_o)
                nc.gpsimd.dma_start(outstat[p][64:128, jc0:jc0 + QCHUNK], ot)

            def emit_oproj(mh, oprojps):
                """o_proj for m-columns [512*mh, 512*(mh+1))."""
                mcol = 512 * mh
                for nt in range(KT):
                    ps = oprojps.tile([128, 512], F32, tag="o",
                                      name=f"ops{nt}_{mh}")
                    for p in range(2):
                        nc.tensor.matmul(
                            ps, wo_sb[p][:, 128 * nt:128 * (nt + 1)],
                            outstat[p][:, mcol:mcol + 512],
                            start=(p == 0), stop=(p == 1))
                    ysb = ysbp.tile([128, 512], BF16, tag="y",
                                    name=f"ysb{nt}_{mh}")
                    if (nt + mh) % 2 == 0:
                        nc.vector.tensor_copy(ysb, ps)
                    else:
                        nc.scalar.copy(ysb, ps)
                    eng = nc.sync if nt % 2 == 0 else nc.gpsimd
                    eng.dma_start(
                        yT[128 * nt:128 * (nt + 1), mcol:mcol + 512], ysb)

            with tc.tile_pool(name="qkvps", bufs=2, space="PSUM") as qkvps:
                emit_qkv_chunk(0, qkvps)
                emit_qkv_chunk(1, qkvps)
                emit_attn(0, 0)
                emit_qkv_chunk(2, qkvps)
                emit_qkv_chunk(3, qkvps)
                emit_attn(0, 1)
            with tc.tile_pool(name="ops", bufs=2, space="PSUM") as oprojps:
                emit_attn(1, 0)
                emit_oproj(0, oprojps)
                emit_oproj(1, oprojps)
                emit_attn(1, 1)
                emit_oproj(2, oprojps)
                emit_oproj(3, oprojps)

    nc.compile()
    return nc


def make_host_inputs(x, w_qkv, w_o):
    """Host-side prep: transpose x, per-core weight slices, rope tables."""
    x = np.asarray(x, dtype=np.float32)
    w_qkv = np.asarray(w_qkv, dtype=np.float32)
    w_o = np.asarray(w_o, dtype=np.float32)
    xT = np.ascontiguousarray(x.reshape(S, HID).T).astype(NPBF)

    inv_freq = 1.0 / (ROPE_BASE ** (np.arange(0, HD, 2, dtype=np.float32) / HD))
    t = np.arange(S, dtype=np.float32)
    freqs = np.outer(t, inv_freq)                     # [S, 32]
    cosT = np.cos(freqs).T.astype(np.float32)         # [32, S]
    sinT = np.sin(freqs).T.astype(np.float32)
    C = np.tile(cosT, (4, 1)).astype(NPBF)            # [128, S]
    Sn = np.tile(np.concatenate([-sinT, sinT], 0), (2, 1)).astype(NPBF)

    r = np.arange(128)
    maskneg = np.where(r[None, :] < r[:, None], np.float32(NEG),
                       np.float32(0.0)).astype(np.float32)

    in_maps = []
    for c in range(NCORES):
        qcols = np.arange(4 * c * HD, 4 * (c + 1) * HD)
        vcols = NH * HD + NKV * HD + np.arange(c * HD, (c + 1) * HD)
        kcols = NH * HD + np.arange(c * HD, (c + 1) * HD)
        w_stat = np.ascontiguousarray(
            np.concatenate([w_qkv[:, qcols], w_qkv[:, vcols], w_qkv[:, kcols]],
                           axis=1)).astype(NPBF)
        w_o_c = np.ascontiguousarray(
            w_o[256 * c:256 * (c + 1), :]).astype(NPBF)
        in_maps.append({
            "xT": xT, "w_stat": w_stat, "w_o": w_o_c,
            "C": C, "Sn": Sn, "maskneg": maskneg,
        })
    return in_maps


_NC_CACHE = {}


def get_nc():
    if "nc" not in _NC_CACHE:
        _NC_CACHE["nc"] = build_kernel()
    return _NC_CACHE["nc"]


def _get_exec():
    """Build (once) the jitted sharded executable over the 8 cores."""
    if "exec" in _NC_CACHE:
        return _NC_CACHE["exec"]
    import jax
    from jax.sharding import Mesh, PartitionSpec, NamedSharding
    from jax.experimental.shard_map import shard_map
    from concourse import bass2jax

    nc = get_nc()
    bass2jax.install_neuronx_cc_hook()
    partition_name = (nc.partition_id_tensor.name
                      if nc.partition_id_tensor else None)
    in_names, out_names, out_avals, zero_outs = [], [], [], []
    for alloc in nc.m.functions[0].allocations:
        if not isinstance(alloc, mybir.MemoryLocationSet):
            continue
        name = alloc.memorylocations[0].name
        if alloc.kind == "ExternalInput":
            if name != partition_name:
                in_names.append(name)
        elif alloc.kind == "ExternalOutput":
            shape = tuple(alloc.tensor_shape)
            dtype = mybir.dt.np(alloc.dtype)
            out_names.append(name)
            out_avals.append(jax.core.ShapedArray(shape, dtype))
            zero_outs.append(np.zeros(shape, dtype))
    n_params = len(in_names)
    all_in = list(in_names) + list(out_names)
    if partition_name is not None:
        all_in.append(partition_name)

    def _body(*args):
        operands = list(args)
        if partition_name is not None:
            operands.append(bass2jax.partition_id_tensor())
        return tuple(bass2jax._bass_exec_p.bind(
            *operands, out_avals=tuple(out_avals), in_names=tuple(all_in),
            out_names=tuple(out_names), lowering_input_output_aliases=(),
            sim_require_finite=True, sim_require_nnan=True, nc=nc))

    devices = jax.devices()[:NCORES]
    mesh = Mesh(np.asarray(devices), ("core",))
    # per-core inputs are sharded over "core"; replicated inputs broadcast
    REPL = {"xT", "C", "Sn", "maskneg"}
    in_specs = tuple(PartitionSpec() if n in REPL else PartitionSpec("core")
                     for n in in_names)
    in_specs = in_specs + (PartitionSpec("core"),) * len(out_names)
    f = jax.jit(shard_map(_body, mesh=mesh, in_specs=in_specs,
                          out_specs=(PartitionSpec("core"),) * len(out_names),
                          check_rep=False), keep_unused=True)
    sh = NamedSharding(mesh, PartitionSpec("core"))
    shr = NamedSharding(mesh, PartitionSpec())
    _NC_CACHE["exec"] = (f, in_names, out_names, zero_outs, sh, shr, REPL)
    return _NC_CACHE["exec"]


def kernel(x, w_qkv, w_o):
    import jax

    f, in_names, out_names, zero_outs, sh, shr, REPL = _get_exec()
    in_maps = make_host_inputs(x, w_qkv, w_o)
    args = []
    for name in in_names:
        if name in REPL:
            args.append(jax.device_put(in_maps[0][name], shr))
        else:
            args.append(jax.device_put(
                np.concatenate([m[name] for m in in_maps], 0), sh))
    if "zeros" not in _NC_CACHE:
        _NC_CACHE["zeros"] = [
            jax.device_put(
                np.zeros((NCORES * z.shape[0], *z.shape[1:]), z.dtype), sh)
            for z in zero_outs]
    args += _NC_CACHE["zeros"]
    outs = f(*args)
    y_idx = out_names.index("yT")
    if "reduce" not in _NC_CACHE:
        import jax.numpy as jnp
        _NC_CACHE["reduce"] = jax.jit(
            lambda a: jnp.transpose(jnp.sum(
                jnp.reshape(a.astype(jnp.float32), (NCORES, HID, S)), axis=0)))
    out = np.asarray(_NC_CACHE["reduce"](outs[y_idx]))
    return np.ascontiguousarray(out.astype(np.float32)).reshape(B, S, HID)


# revision 15
# speedup vs baseline: 2.1205x; 2.1205x over previous
"""GQA causal attention layer (QKV proj + NeoX RoPE + softmax attention + o_proj)
for Trainium2, tensor-parallel over heads across 8 NeuronCores.

Problem shapes (hardcoded): B=1, S=2048, HID=2048, NH=32, NKV=8, HD=64.
Per core c: 4 query heads (4c..4c+3) + 1 kv head (c).

v2 (bf16, packed scores, pipelined):
  - All SBUF operands bf16 (PSUM accumulation f32): halves DMA, enables FWL
    weight loads and 2x DVE modes. Accuracy ~0.3% << 2e-2 gate.
  - GQA: all 4 q heads share one kv head, so score matmuls for the even head
    (q rows 0:64, k dup rows 0:64) and odd head (rows 64:128) are emitted as
    adjacent 64x128-mode matmuls at row-tile positions (0,0)/(64,0) -> they
    execute CONCURRENTLY on the PE array (2 cols/cycle aggregate).
  - v transposed via XBAR DMA transpose (no PE transposes, no extra PSUM).
  - Single interleaved emission: qkv chunks, attention (j, head-pair) blocks,
    and o_proj m-chunks share one dependency-scheduled stream so the ~60us
    of ACT exp work hides under PE work. PSUM: ste(2)+sto(2)+pv(2)+qkv/o(2).
  - Per (j, pair): even head PV accumulates in the i-loop (pt ring);
    odd head PV replays from saved pt tiles through the same psum ring.

Host transposes x, pre-slices per-core weights, converts to bf16, and sums
the 8 partial yT outputs (o_proj contraction is split across cores).
"""

import numpy as np
import ml_dtypes

import concourse.bass as bass
import concourse.mybir as mybir
import concourse.tile as tile
from concourse import bacc

B, S, HID = 1, 2048, 2048
NH, NKV, HD = 32, 8, 64
NCORES = 8
ROPE_BASE = 10000.0
SCALE = 1.0 / np.sqrt(HD)   # 0.125
NEG = -1e9

F32 = mybir.dt.float32
BF16 = mybir.dt.bfloat16
NPBF = ml_dtypes.bfloat16

KT = S // 128               # 16 k-position tiles of 128
MC = 512                    # qkv m-chunk
NMC = S // MC               # 4
QCHUNK = 1024               # attention q-chunk
NQC = S // QCHUNK           # 2
EXPF = mybir.ActivationFunctionType.Exp


def _chunks(total, step=512):
    out = []
    o = 0
    while o < total:
        out.append((o, min(step, total - o)))
        o += step
    return out


def build_kernel(passes=1):
    nc = bacc.Bacc("TRN2", target_bir_lowering=False, debug=False,
                   num_devices=NCORES)

    xT = nc.dram_tensor("xT", [HID, S], BF16, kind="ExternalInput").ap()
    w_stat = nc.dram_tensor("w_stat", [HID, 384], BF16, kind="ExternalInput").ap()
    w_o = nc.dram_tensor("w_o", [256, HID], BF16, kind="ExternalInput").ap()
    Cr = nc.dram_tensor("C", [128, S], BF16, kind="ExternalInput").ap()
    Sr = nc.dram_tensor("Sn", [128, S], BF16, kind="ExternalInput").ap()
    mask01 = nc.dram_tensor("mask01", [128, 128], BF16, kind="ExternalInput").ap()
    yT = nc.dram_tensor("yT", [HID, S], BF16, kind="ExternalOutput").ap()

    with tile.TileContext(nc) as tc:
      for _pass in range(passes):
        with (
            tc.tile_pool(name="pers", bufs=1) as pers,
            tc.tile_pool(name="wtp", bufs=KT) as wtp,
            tc.tile_pool(name="vaugp", bufs=1) as vaugp,
            tc.tile_pool(name="xp", bufs=24) as xp,
            tc.tile_pool(name="qkvsb", bufs=2) as qkvsb,
            tc.tile_pool(name="swp", bufs=2) as swp,
            tc.tile_pool(name="rtmp", bufs=2) as rtmp,
            tc.tile_pool(name="ptAp", bufs=4) as ptAp,
            tc.tile_pool(name="ptBp", bufs=KT) as ptBp,
            tc.tile_pool(name="recp", bufs=2) as recp,
            tc.tile_pool(name="otp", bufs=2) as otp,
            tc.tile_pool(name="ysbp", bufs=4) as ysbp,
            tc.tile_pool(name="stps", bufs=1, space="PSUM") as stps,
            tc.tile_pool(name="pvps", bufs=1, space="PSUM") as pvps,
        ):
            # ---- persistent tiles ----
            qr = [pers.tile([128, S], BF16, tag=f"qr{p}", name=f"qr{p}")
                  for p in range(2)]
            # kr rows 64:128 = roped kT; rows 0:64 = DMA duplicate (even heads)
            kr = pers.tile([128, S], BF16, tag="kr")
            outstat = [pers.tile([128, S], BF16, tag=f"os{p}", name=f"os{p}")
                       for p in range(2)]
            wo_sb = [pers.tile([128, HID], BF16, tag=f"wo{p}", name=f"wo{p}")
                     for p in range(2)]
            m01 = pers.tile([128, 128], BF16, tag="m01")
            Ct = pers.tile([128, S], BF16, tag="Ct")
            St = pers.tile([128, S], BF16, tag="St")
            vaug = [vaugp.tile([128, 128], BF16, tag=f"va{i}", name=f"va{i}")
                    for i in range(KT)]

            nc.scalar.dma_start(m01, mask01)
            nc.scalar.dma_start(Ct, Cr)
            nc.scalar.dma_start(St, Sr)
            for p in range(2):
                nc.scalar.dma_start(wo_sb[p], w_o[128 * p:128 * (p + 1), :])
            for i in range(KT):
                nc.gpsimd.memset(vaug[i][:, 64:128], 1.0)
            wt = []
            for k in range(KT):
                w = wtp.tile([128, 384], BF16, tag="w", name=f"w{k}")
                nc.gpsimd.dma_start(w, w_stat[128 * k:128 * (k + 1), :])
                wt.append(w)

            def emit_qkv_chunk(c, qkvps):
                """w_stat.T @ x for m-chunk c, + RoPE into qr/kr, + v->vaug."""
                m0 = MC * c
                xts = []
                for k in range(KT):
                    xt = xp.tile([128, MC], BF16, tag="x", name=f"x{c}_{k}")
                    eng = nc.sync if k % 2 == 0 else nc.scalar
                    eng.dma_start(xt, xT[128 * k:128 * (k + 1), m0:m0 + MC])
                    xts.append(xt)
                for n in range(3):
                    ps = qkvps.tile([128, MC], F32, tag="qkv",
                                    name=f"qkvps{c}_{n}")
                    for k in range(KT):
                        nc.tensor.matmul(ps, wt[k][:, 128 * n:128 * (n + 1)],
                                         xts[k], start=(k == 0),
                                         stop=(k == KT - 1))
                    qn = qkvsb.tile([128, MC], BF16, tag=f"q{n}",
                                    name=f"qn{c}_{n}")
                    if n % 2 == 0:
                        nc.vector.tensor_copy(qn, ps)
                    else:
                        nc.scalar.copy(qn, ps)
                    # RoPE (NeoX rotate-halves via 32-row swap + cos/sin mulsum)
                    r0, r1 = (0, 128) if n < 2 else (64, 128)
                    sw = swp.tile([128, MC], BF16, tag="sw", name=f"sw{c}_{n}")
                    for g in range(r0 // 32, r1 // 32, 2):
                        nc.gpsimd.dma_start(sw[32 * g:32 * g + 32, :],
                                            qn[32 * g + 32:32 * g + 64, :])
                        nc.gpsimd.dma_start(sw[32 * g + 32:32 * g + 64, :],
                                            qn[32 * g:32 * g + 32, :])
                    t1 = rtmp.tile([128, MC], BF16, tag="t1", name=f"t1_{c}{n}")
                    t2 = rtmp.tile([128, MC], BF16, tag="t2", name=f"t2_{c}{n}")
                    nc.vector.tensor_mul(t1[r0:r1, :], qn[r0:r1, :],
                                         Ct[r0:r1, m0:m0 + MC])
                    nc.vector.tensor_mul(t2[r0:r1, :], sw[r0:r1, :],
                                         St[r0:r1, m0:m0 + MC])
                    dst = qr[n] if n < 2 else kr
                    nc.vector.tensor_add(dst[r0:r1, m0:m0 + MC],
                                         t1[r0:r1, :], t2[r0:r1, :])
                    if n == 2:
                        nc.gpsimd.dma_start(kr[0:64, m0:m0 + MC],
                                            kr[64:128, m0:m0 + MC])
                        for ii in range(4 * c, 4 * (c + 1)):
                            off = 128 * ii - m0
                            nc.sync.dma_start_transpose(
                                vaug[ii][:, 0:64], qn[0:64, off:off + 128])

            def emit_attn(j, p):
                """Attention q-chunk j for head pair p (heads 2p, 2p+1)."""
                jc0 = QCHUNK * j
                ilast = 8 * (j + 1) - 1
                pv = pvps.tile([128, QCHUNK], F32, tag="pv", name=f"pve{j}_{p}")
                ptBs = []
                for i in range(8 * (j + 1)):
                    qstart = max(jc0, 128 * i)
                    qlen = QCHUNK * (j + 1) - qstart
                    qoff = qstart - jc0
                    ste = stps.tile([128, QCHUNK], F32, tag="ste",
                                    name=f"ste{j}_{p}_{i}")
                    sto = stps.tile([128, QCHUNK], F32, tag="sto",
                                    name=f"sto{j}_{p}_{i}")
                    for (c0, cl) in _chunks(qlen):
                        nc.tensor.matmul(
                            ste[:, c0:c0 + cl],
                            kr[0:64, 128 * i:128 * (i + 1)],
                            qr[p][0:64, qstart + c0:qstart + c0 + cl],
                            start=True, stop=True)
                        nc.tensor.matmul(
                            sto[:, c0:c0 + cl],
                            kr[64:128, 128 * i:128 * (i + 1)],
                            qr[p][64:128, qstart + c0:qstart + c0 + cl],
                            start=True, stop=True)
                    ptA = ptAp.tile([128, QCHUNK], BF16, tag="ptA",
                                    name=f"ptA{j}_{p}_{i}")
                    ptB = ptBp.tile([128, QCHUNK], BF16, tag="ptB",
                                    name=f"ptB{j}_{p}_{i}")
                    nc.scalar.activation(ptA[:, 0:qlen], ste[:, 0:qlen],
                                         EXPF, scale=SCALE)
                    nc.scalar.activation(ptB[:, 0:qlen], sto[:, 0:qlen],
                                         EXPF, scale=SCALE)
                    if 128 * i >= jc0:
                        # causal trim of the diagonal 128x128 block: cheap
                        # all-SBUF bf16 multiply by a 0/1 mask (post-exp)
                        nc.vector.tensor_mul(ptA[:, 0:128], ptA[:, 0:128], m01)
                        nc.vector.tensor_mul(ptB[:, 0:128], ptB[:, 0:128], m01)
                    ptBs.append((ptB, qoff, qlen))
                    for (c0, cl) in _chunks(qlen):
                        nc.tensor.matmul(
                            pv[:, qoff + c0:qoff + c0 + cl],
                            vaug[i], ptA[:, c0:c0 + cl],
                            start=(i == 0), stop=(i == ilast))
                # normalize even head -> outstat[p] rows 0:64
                rc = recp.tile([128, QCHUNK], F32, tag="rc", name=f"rch{j}_{p}")
                nc.scalar.copy(rc[64:128, :], pv[64:128, :])
                sums = recp.tile([64, QCHUNK], F32, tag="sums",
                                 name=f"sme{j}_{p}")
                nc.gpsimd.dma_start(sums, rc[64:128, :])
                rec0 = recp.tile([64, QCHUNK], F32, tag="rec",
                                 name=f"rce{j}_{p}")
                nc.vector.reciprocal_approx_fast(rec0, sums)
                nc.vector.tensor_mul(outstat[p][0:64, jc0:jc0 + QCHUNK],
                                     pv[0:64, :], rec0)
                # odd head PV replay from saved pt tiles (same psum ring)
                pvo = pvps.tile([128, QCHUNK], F32, tag="pv", name=f"pvo{j}_{p}")
                for i, (ptB, qoff, qlen) in enumerate(ptBs):
                    for (c0, cl) in _chunks(qlen):
                        nc.tensor.matmul(
                            pvo[:, qoff + c0:qoff + c0 + cl],
                            vaug[i], ptB[:, c0:c0 + cl],
                            start=(i == 0), stop=(i == ilast))
                rco = recp.tile([128, QCHUNK], F32, tag="rc",
                                name=f"rcho{j}_{p}")
                nc.scalar.copy(rco[64:128, :], pvo[64:128, :])
                sums_o = recp.tile([64, QCHUNK], F32, tag="sums",
                                   name=f"smo{j}_{p}")
                nc.gpsimd.dma_start(sums_o, rco[64:128, :])
                rec_o = recp.tile([64, QCHUNK], F32, tag="rec",
                                  name=f"rco{j}_{p}")
                nc.vector.reciprocal_approx_fast(rec_o, sums_o)
                ot = otp.tile([64, QCHUNK], BF16, tag="ot", name=f"ot{j}_{p}")
                nc.vector.tensor_mul(ot, pvo[0:64, :], rec_o)
                nc.gpsimd.dma_start(outstat[p][64:128, jc0:jc0 + QCHUNK], ot)

            def emit_oproj(mh, oprojps):
                """o_proj for m-columns [512*mh, 512*(mh+1))."""
                mcol = 512 * mh
                for nt in range(KT):
                    ps = oprojps.tile([128, 512], F32, tag="o",
                                      name=f"ops{nt}_{mh}")
                    for p in range(2):
                        nc.tensor.matmul(
                            ps, wo_sb[p][:, 128 * nt:128 * (nt + 1)],
                            outstat[p][:, mcol:mcol + 512],
                            start=(p == 0), stop=(p == 1))
                    ysb = ysbp.tile([128, 512], BF16, tag="y",
                                    name=f"ysb{nt}_{mh}")
                    if (nt + mh) % 2 == 0:
                        nc.vector.tensor_copy(ysb, ps)
                    else:
                        nc.scalar.copy(ysb, ps)
                    eng = nc.sync if nt % 2 == 0 else nc.gpsimd
                    eng.dma_start(
                        yT[128 * nt:128 * (nt + 1), mcol:mcol + 512], ysb)

            with tc.tile_pool(name="qkvps", bufs=2, space="PSUM") as qkvps:
                emit_qkv_chunk(0, qkvps)
                emit_qkv_chunk(1, qkvps)
                emit_attn(0, 0)
                emit_qkv_chunk(2, qkvps)
                emit_qkv_chunk(3, qkvps)
                emit_attn(0, 1)
            with tc.tile_pool(name="ops", bufs=2, space="PSUM") as oprojps:
                emit_attn(1, 0)
                emit_oproj(0, oprojps)
                emit_oproj(1, oprojps)
                emit_attn(1, 1)
                emit_oproj(2, oprojps)
                emit_oproj(3, oprojps)

    nc.compile()
    return nc


def make_host_inputs(x, w_qkv, w_o):
    """Host-side prep: transpose x, per-core weight slices, rope tables."""
    x = np.asarray(x, dtype=np.float32)
    w_qkv = np.asarray(w_qkv, dtype=np.float32)
    w_o = np.asarray(w_o, dtype=np.float32)
    xT = np.ascontiguousarray(x.reshape(S, HID).T).astype(NPBF)

    inv_freq = 1.0 / (ROPE_BASE ** (np.arange(0, HD, 2, dtype=np.float32) / HD))
    t = np.arange(S, dtype=np.float32)
    freqs = np.outer(t, inv_freq)                     # [S, 32]
    cosT = np.cos(freqs).T.astype(np.float32)         # [32, S]
    sinT = np.sin(freqs).T.astype(np.float32)
    C = np.tile(cosT, (4, 1)).astype(NPBF)            # [128, S]
    Sn = np.tile(np.concatenate([-sinT, sinT], 0), (2, 1)).astype(NPBF)

    r = np.arange(128)
    mask01 = np.where(r[None, :] < r[:, None], np.float32(0.0),
                      np.float32(1.0)).astype(NPBF)

    in_maps = []
    for c in range(NCORES):
        qcols = np.arange(4 * c * HD, 4 * (c + 1) * HD)
        vcols = NH * HD + NKV * HD + np.arange(c * HD, (c + 1) * HD)
        kcols = NH * HD + np.arange(c * HD, (c + 1) * HD)
        w_stat = np.ascontiguousarray(
            np.concatenate([w_qkv[:, qcols], w_qkv[:, vcols], w_qkv[:, kcols]],
                           axis=1)).astype(NPBF)
        w_o_c = np.ascontiguousarray(
            w_o[256 * c:256 * (c + 1), :]).astype(NPBF)
        in_maps.append({
            "xT": xT, "w_stat": w_stat, "w_o": w_o_c,
            "C": C, "Sn": Sn, "mask01": mask01,
        })
    return in_maps


_NC_CACHE = {}


def get_nc():
    if "nc" not in _NC_CACHE:
        _NC_CACHE["nc"] = build_kernel()
    return _NC_CACHE["nc"]


def _get_exec():
    """Build (once) the jitted sharded executable over the 8 cores."""
    if "exec" in _NC_CACHE:
        return _NC_CACHE["exec"]
    import jax
    from jax.sharding import Mesh, PartitionSpec, NamedSharding
    from jax.experimental.shard_map import shard_map
    from concourse import bass2jax

    nc = get_nc()
    bass2jax.install_neuronx_cc_hook()
    partition_name = (nc.partition_id_tensor.name
                      if nc.partition_id_tensor else None)
    in_names, out_names, out_avals, zero_outs = [], [], [], []
    for alloc in nc.m.functions[0].allocations:
        if not isinstance(alloc, mybir.MemoryLocationSet):
            continue
        name = alloc.memorylocations[0].name
        if alloc.kind == "ExternalInput":
            if name != partition_name:
                in_names.append(name)
        elif alloc.kind == "ExternalOutput":
            shape = tuple(alloc.tensor_shape)
            dtype = mybir.dt.np(alloc.dtype)
            out_names.append(name)
            out_avals.append(jax.core.ShapedArray(shape, dtype))
            zero_outs.append(np.zeros(shape, dtype))
    n_params = len(in_names)
    all_in = list(in_names) + list(out_names)
    if partition_name is not None:
        all_in.append(partition_name)

    def _body(*args):
        operands = list(args)
        if partition_name is not None:
            operands.append(bass2jax.partition_id_tensor())
        return tuple(bass2jax._bass_exec_p.bind(
            *operands, out_avals=tuple(out_avals), in_names=tuple(all_in),
            out_names=tuple(out_names), lowering_input_output_aliases=(),
            sim_require_finite=True, sim_require_nnan=True, nc=nc))

    devices = jax.devices()[:NCORES]
    mesh = Mesh(np.asarray(devices), ("core",))
    # per-core inputs are sharded over "core"; replicated inputs broadcast
    REPL = {"xT", "C", "Sn", "mask01"}
    in_specs = tuple(PartitionSpec() if n in REPL else PartitionSpec("core")
                     for n in in_names)
    in_specs = in_specs + (PartitionSpec("core"),) * len(out_names)
    f = jax.jit(shard_map(_body, mesh=mesh, in_specs=in_specs,
                          out_specs=(PartitionSpec("core"),) * len(out_names),
                          check_rep=False), keep_unused=True)
    sh = NamedSharding(mesh, PartitionSpec("core"))
    shr = NamedSharding(mesh, PartitionSpec())
    _NC_CACHE["exec"] = (f, in_names, out_names, zero_outs, sh, shr, REPL)
    return _NC_CACHE["exec"]


def kernel(x, w_qkv, w_o):
    import jax

    f, in_names, out_names, zero_outs, sh, shr, REPL = _get_exec()
    in_maps = make_host_inputs(x, w_qkv, w_o)
    args = []
    for name in in_names:
        if name in REPL:
            args.append(jax.device_put(in_maps[0][name], shr))
        else:
            args.append(jax.device_put(
                np.concatenate([m[name] for m in in_maps], 0), sh))
    if "zeros" not in _NC_CACHE:
        _NC_CACHE["zeros"] = [
            jax.device_put(
                np.zeros((NCORES * z.shape[0], *z.shape[1:]), z.dtype), sh)
            for z in zero_outs]
    args += _NC_CACHE["zeros"]
    outs = f(*args)
    y_idx = out_names.index("yT")
    if "reduce" not in _NC_CACHE:
        import jax.numpy as jnp
        _NC_CACHE["reduce"] = jax.jit(
            lambda a: jnp.transpose(jnp.sum(
                jnp.reshape(a.astype(jnp.float32), (NCORES, HID, S)), axis=0)))
    out = np.asarray(_NC_CACHE["reduce"](outs[y_idx]))
    return np.ascontiguousarray(out.astype(np.float32)).reshape(B, S, HID)
